# revision 39
# baseline (speedup 1.0000x reference)
"""Trainium2 Bass kernel for nn_LocalPlasticityNet (FFT front-end + Hebbian MLP).

Data-parallel over batch: 8 cores x 128 rows. Per core:
  FFT(20000) as four-step DFT, 20000 = 125*160, n = n1 + 125*n2,
  k = k2 + 160*k1 (k1 < 63 covers the needed half spectrum).
    stage A: G[n1,k2]  = sum_n2 xt[n2,n1] W160^{n2 k2}   (per-sample MMs, f16)
    stage B: Bt = T' * G twiddle                         (DVE + GPSIMD, f16)
    stage C: per k2-chunk, 2 MMs with packed stationaries
             W1=[c|0|-s], W2=[s|0|c] -> PSUM [Xr(0:63); Xi(64:127)]
  magnitude: ACT Square -> DVE add -> DVE pow(.5) -> ACT log1p  (single
  act-table region; chunk-PAIRS stacked on partitions 0:63 / 63:126)
  layer 0 streams W0/A0 (f16, host-permuted to the pair layout); the norm
  mask rides as column 256 of the A0 stream so sum(mask*h^2) accumulates in
  PSUM alongside the Hebbian trace. rsqrt everywhere via DVE pow(-0.5) (no
  sqrt-table loads). Layers 1/2 + head in f16/f32 on-chip.
x is host-side cast to f16 and pre-transposed to [n2, b, n1] so the DMA is
contiguous. Mean subtraction == zeroing the k=0 bin (W0/A0/mask rows f=0).
"""
import numpy as np
import ml_dtypes

import concourse.bass as bass
import concourse.tile as tile
import concourse.mybir as mybir
from concourse import bacc
from concourse.bass_utils import run_bass_kernel_spmd

AF = mybir.ActivationFunctionType
ALU = mybir.AluOpType
F32 = mybir.dt.float32
F32R = mybir.dt.float32r
F16 = mybir.dt.float16
BF16 = mybir.dt.bfloat16

B, N, NF = 1024, 20000, 10001
N1, N2 = 125, 160          # n = n1 + 125*n2
K1, K2 = 63, 160           # k = k2 + 160*k1
NCORES = 8
BL = B // NCORES           # 128
H0, H1, H2 = 256, 128, 64
LN_EPS = 1e-5
GB = 4                     # stage A batch group (psum banks)
XBG = 32                   # x streamed in batch groups of 32
CK = 4                     # stage C k2-chunk width
NPAIR = K2 // (2 * CK)     # 20 chunk pairs
WG = 2                     # W0 stream group: WG pairs per DMA

_cache = {}


def _f16(x):
    return np.ascontiguousarray(np.asarray(x, dtype=ml_dtypes.float16
                                           if hasattr(ml_dtypes, "float16")
                                           else np.float16))


def _f32(x):
    return np.ascontiguousarray(np.asarray(x, dtype=np.float32))


def build_consts(inputs):
    n2 = np.arange(N2)
    k2 = np.arange(K2)
    n1 = np.arange(N1)
    k1 = np.arange(K1)
    c = {}
    # stage A rhs [160, 320] = [cos | -sin] of 2pi n2 k2/160, split 128+32
    ang = 2 * np.pi * np.outer(n2, k2) / N2
    f160 = np.concatenate([np.cos(ang), -np.sin(ang)], axis=1)
    c["f160h"] = _f16(f160[0:128])
    c["f160l"] = _f16(f160[128:160])
    # twiddle [125, 160], replicated x8 over the sample dim so the DVE ops
    # see unit-stride operands (stride-0 broadcasts can defeat 2x packing)
    angt = 2 * np.pi * np.outer(n1, k2) / N
    c["tc"] = _f16(np.repeat(np.cos(angt)[:, None, :], 8, axis=1))
    c["ts"] = _f16(np.repeat(np.sin(angt)[:, None, :], 8, axis=1))
    # stage C packed stationaries [125, 128]
    angc = 2 * np.pi * np.outer(n1, k1) / N1
    cc_ = np.cos(angc)
    ss_ = np.sin(angc)
    W1s = np.zeros((N1, 128))
    W1s[:, 0:63] = cc_
    W1s[:, 64:127] = -ss_
    W2s = np.zeros((N1, 128))
    W2s[:, 0:63] = ss_
    W2s[:, 64:127] = cc_
    c["W1s"] = _f16(W1s)
    c["W2s"] = _f16(W2s)
    # PE fold matrices: m2pair[c] = sq[c] + sq[64+c] (even chunk -> cols 0:63,
    # odd chunk -> cols 63:126)
    fe = np.zeros((128, 126))
    fo = np.zeros((128, 126))
    for r in range(63):
        fe[r, r] = 1.0
        fe[64 + r, r] = 1.0
        fo[r, 63 + r] = 1.0
        fo[64 + r, 63 + r] = 1.0
    c["folde"] = np.ascontiguousarray(fe.astype(ml_dtypes.bfloat16))
    c["foldo"] = np.ascontiguousarray(fo.astype(ml_dtypes.bfloat16))
    c["ident"] = _f32(np.eye(128))
    # layer 0 weights, permuted to the pair layout:
    # col c = 4t+j ; rows 0:63 -> f = 8t+j+160*k1 ; rows 63:126 -> f = 8t+4+j+160*k1
    W0 = np.asarray(inputs["W0"], np.float64)         # (256, 10001)
    al0 = np.asarray(inputs["alpha0"], np.float64)
    eta0 = float(inputs["eta0"])
    FP = K1 * K2
    W0p = np.zeros((FP, H0))
    W0p[:NF] = W0.T
    A0p = np.zeros((FP, H0))
    A0p[:NF] = (eta0 / (1.0 + np.exp(-al0))).T
    W0p[0] = 0.0          # f=0 bin acts as zero (mean subtraction)
    A0p[0] = 0.0
    maskf = np.zeros(FP)
    maskf[1:NF] = 1.0
    cs = np.arange(K2 // 2)
    tt, jj = cs // CK, cs % CK
    rr = np.arange(K1)
    f_top = (8 * tt + jj)[None, :] + 160 * rr[:, None]        # [63, 80]
    f_bot = (8 * tt + 4 + jj)[None, :] + 160 * rr[:, None]
    w0stk = np.zeros((126, K2 // 2, H0))
    w0stk[0:63] = W0p[f_top]
    w0stk[63:126] = W0p[f_bot]
    a0stk = np.zeros((126, K2 // 2, H0 + 1))
    a0stk[0:63, :, 0:H0] = A0p[f_top]
    a0stk[63:126, :, 0:H0] = A0p[f_bot]
    a0stk[0:63, :, H0] = maskf[f_top]
    a0stk[63:126, :, H0] = maskf[f_bot]
    c["w0stk"] = _f16(w0stk)
    c["a0stk"] = _f16(a0stk)
    c["b0b"] = _f32(np.tile(np.asarray(inputs["b0"], np.float32), (BL, 1)))
    # small layers: transposed [f, h] f16
    W1 = np.asarray(inputs["W1"], np.float64)         # (128, 256)
    A1 = float(inputs["eta1"]) / (1.0 + np.exp(-np.asarray(inputs["alpha1"], np.float64)))
    c["w1t"] = _f16(W1.T.reshape(2, 128, H1).transpose(1, 0, 2))   # [128, 2, 128]
    c["a1t"] = _f16(A1.T.reshape(2, 128, H1).transpose(1, 0, 2))
    W2 = np.asarray(inputs["W2"], np.float64)         # (64, 128)
    A2 = float(inputs["eta2"]) / (1.0 + np.exp(-np.asarray(inputs["alpha2"], np.float64)))
    c["w2t"] = _f16(W2.T)                                          # [128, 64]
    c["a2t"] = _f16(A2.T)
    hw = np.zeros((H2, 2))
    hw[:, 0] = np.asarray(inputs["head_w"], np.float64)[0]
    c["hwt"] = _f16(hw)    # [64, 2], col 1 zero-pad
    for nm, h in (("0", H0), ("1", H1), ("2", H2)):
        c[f"g{nm}b"] = _f32(np.tile(np.asarray(inputs[f"g{nm}"], np.float32), (BL, 1)))
        c[f"be{nm}b"] = _f32(np.tile(np.asarray(inputs[f"be{nm}"], np.float32), (BL, 1)))
    c["b1b"] = _f32(np.tile(np.asarray(inputs["b1"], np.float32), (BL, 1)))
    c["b2b"] = _f32(np.tile(np.asarray(inputs["b2"], np.float32), (BL, 1)))
    c["hbb"] = _f32(np.tile(np.asarray(inputs["head_b"], np.float32).reshape(1), (BL, 1)))
    return c


CONST_DTYPES = {
    "f160h": F16, "f160l": F16, "tc": F16, "ts": F16,
    "W1s": F16, "W2s": F16, "folde": BF16, "foldo": BF16, "ident": F32R,
    "w0stk": F16, "a0stk": F16, "b0b": F32,
    "w1t": F16, "a1t": F16, "w2t": F16, "a2t": F16, "hwt": F16,
    "g0b": F32, "be0b": F32, "g1b": F32, "be1b": F32, "g2b": F32, "be2b": F32,
    "b1b": F32, "b2b": F32, "hbb": F32,
}

SHAPES = {
    "f160h": [128, 2 * K2], "f160l": [32, 2 * K2],
    "tc": [N1, 8, K2], "ts": [N1, 8, K2],
    "W1s": [N1, 128], "W2s": [N1, 128],
    "folde": [128, 126], "foldo": [128, 126], "ident": [128, 128],
    "w0stk": [126, K2 // 2, H0], "a0stk": [126, K2 // 2, H0 + 1], "b0b": [BL, H0],
    "w1t": [128, 2, H1], "a1t": [128, 2, H1],
    "w2t": [H1, H2], "a2t": [H1, H2], "hwt": [H2, 2],
    "g0b": [BL, H0], "be0b": [BL, H0], "g1b": [BL, H1], "be1b": [BL, H1],
    "g2b": [BL, H2], "be2b": [BL, H2], "b1b": [BL, H1], "b2b": [BL, H2],
    "hbb": [BL, 1],
}


I32 = mybir.dt.int32


def _rsqrt(nc, once, src_ap, tag, add=None):
    """[BL,1] rsqrt via DVE quake-seed + 2 Newton iterations (no act table)."""
    v = once.tile([BL, 1], F32, tag=f"{tag}v")
    if add is not None:
        nc.vector.tensor_scalar_add(v[:], src_ap, add)
    else:
        nc.vector.tensor_copy(v[:], src_ap)
    iv = once.tile([BL, 1], I32, tag=f"{tag}i")
    nc.vector.tensor_scalar(iv[:], v[:].bitcast(I32), 1, None,
                            op0=ALU.logical_shift_right)
    nc.vector.tensor_scalar(iv[:], iv[:], -1, 0x5f3759df,
                            op0=ALU.mult, op1=ALU.add)
    y = once.tile([BL, 1], F32, tag=f"{tag}y")
    nc.vector.tensor_copy(y[:], iv[:].bitcast(F32))
    t1 = once.tile([BL, 1], F32, tag=f"{tag}t")
    for _ in range(2):
        nc.vector.tensor_tensor(t1[:], y[:], y[:], ALU.mult)
        nc.vector.tensor_tensor(t1[:], t1[:], v[:], ALU.mult)
        nc.vector.tensor_scalar(t1[:], t1[:], -0.5, 1.5, op0=ALU.mult, op1=ALU.add)
        nc.vector.tensor_tensor(y[:], y[:], t1[:], ALU.mult)
    return y


def _layer_epilogue(nc, once, y_ps, tr_ps, rinv, H, gb, beb, bb):
    """y = y_slow + tanh(y_slow) * (trace * rinv); LayerNorm(g,be); exact GELU.
    ACT funcs used: Tanh / Gelu / Identity -> one act table; LN stats via
    DVE bn_stats, rstd via DVE pow(-0.5)."""
    ysb = once.tile([BL, H], F32, tag=f"ysb{H}")
    if bb is not None:
        nc.vector.tensor_tensor(ysb[:], y_ps, bb, ALU.add)
    else:
        nc.vector.tensor_copy(ysb[:], y_ps)
    tnh = once.tile([BL, H], F32, tag=f"tnh{H}")
    nc.scalar.activation(tnh[:], ysb[:], AF.Tanh)
    pl = once.tile([BL, H], F32, tag=f"pl{H}")
    nc.vector.scalar_tensor_tensor(pl[:], tr_ps, rinv, tnh[:], ALU.mult, ALU.mult)
    y = once.tile([BL, H], F32, tag=f"y{H}")
    nc.vector.tensor_tensor(y[:], ysb[:], pl[:], ALU.add)
    # LayerNorm via bn_stats (mean+var in one pass)
    stats = once.tile([BL, 6], F32, tag=f"st{H}")
    nc.vector.bn_stats(stats[:], y[:])
    mv = once.tile([BL, 2], F32, tag=f"mv{H}")
    nc.vector.bn_aggr(mv[:], stats[:])
    nmu = once.tile([BL, 1], F32, tag=f"nmu{H}")
    nc.vector.tensor_scalar_mul(nmu[:], mv[:, 0:1], -1.0)
    rstd = _rsqrt(nc, once, mv[:, 1:2], f"rstd{H}", add=LN_EPS)
    ty = once.tile([BL, H], F32, tag=f"ty{H}")
    nc.scalar.activation(ty[:], y[:], AF.Identity, bias=nmu[:])
    yn = once.tile([BL, H], F32, tag=f"yn{H}")
    nc.vector.scalar_tensor_tensor(yn[:], ty[:], rstd[:], gb, ALU.mult, ALU.mult)
    nc.vector.tensor_tensor(yn[:], yn[:], beb, ALU.add)
    hn = once.tile([BL, H], F32R, tag=f"hn{H}")
    nc.scalar.activation(hn[:], yn[:], AF.Gelu)
    return hn


def _norm_rinv(nc, once, h, H):
    """rinv[b,1] ~= 1/||h_row||_2 via Square-accum + DVE pow(-0.5)."""
    d = once.tile([BL, H], F32, tag=f"nsq{H}")
    ns = once.tile([BL, 1], F32, tag=f"nss{H}")
    nc.scalar.activation(d[:], h, AF.Square, accum_out=ns[:])
    return _rsqrt(nc, once, ns[:], f"nrv{H}")


def build_kernel(reps: int = 1):
    nc = bacc.Bacc("TRN2", target_bir_lowering=False, debug=False, num_devices=1)
    xtd = nc.dram_tensor("xt", [N2, BL, N1], F16, kind="ExternalInput").ap()
    cd = {nm: nc.dram_tensor(nm, shp, CONST_DTYPES[nm], kind="ExternalInput").ap()
          for nm, shp in SHAPES.items()}
    out = nc.dram_tensor("out", [BL, 1], F32, kind="ExternalOutput").ap()

    import contextlib
    with tile.TileContext(nc) as tc:
        rep_ctx = tc.For_i(0, reps, 1) if reps > 1 else contextlib.nullcontext()
        with (
            rep_ctx,
            tc.tile_pool(name="const", bufs=1) as cpool,
            tc.tile_pool(name="bt", bufs=1) as btpool,
            tc.tile_pool(name="xt", bufs=2) as xpool,
        ):
            # prefetch the first x block before anything else hits the DMA queue
            xt_hi0 = xpool.tile([128, XBG, N1], F16, tag="xthi")
            xt_lo0 = xpool.tile([32, XBG, N1], F16, tag="xtlo")
            nc.sync.dma_start(xt_hi0[:], xtd[0:128, 0:XBG, :])
            nc.sync.dma_start(xt_lo0[:], xtd[128:160, 0:XBG, :])
            # ---- resident constants (stage-A-critical first) ----
            sb = {}
            for nm in ("f160h", "f160l", "tc", "ts", "W1s", "W2s",
                       "folde", "foldo", "ident",
                       "w1t", "a1t", "w2t", "a2t", "hwt",
                       "g0b", "be0b", "g1b", "be1b", "g2b", "be2b",
                       "b0b", "b1b", "b2b", "hbb"):
                t = cpool.tile(SHAPES[nm], CONST_DTYPES[nm], tag=nm)
                nc.sync.dma_start(t[:], cd[nm])
                sb[nm] = t

            bt = btpool.tile([N1, BL, 2 * K2], F16, tag="bt")  # [n1, b, Btr|Bti]
            btv = bt.rearrange("p b k -> p k b")               # k2-major view
            TB = 2 * GB                                        # twiddle block: 8 samples
            tcb = sb["tc"][:]
            tsb = sb["ts"][:]

            # ---- stage A + twiddle ----
            with (
                tc.tile_pool(name="psum_g", bufs=2, space="PSUM") as pg,
                tc.tile_pool(name="gsb", bufs=3) as gsbpool,
                tc.tile_pool(name="tw", bufs=3) as twpool,
            ):
                for g in range(BL // GB):
                    if g % (XBG // GB) == 0:
                        bg = g * GB // XBG
                        if bg == 0:
                            xt_hi, xt_lo = xt_hi0, xt_lo0
                        else:
                            xt_hi = xpool.tile([128, XBG, N1], F16, tag="xthi")
                            xt_lo = xpool.tile([32, XBG, N1], F16, tag="xtlo")
                            nc.sync.dma_start(xt_hi[:], xtd[0:128, bg * XBG:(bg + 1) * XBG, :])
                            nc.sync.dma_start(xt_lo[:], xtd[128:160, bg * XBG:(bg + 1) * XBG, :])
                    gps = pg.tile([N1, GB, 512], F32, tag="gps")
                    for j in range(GB):
                        bl_ = (g * GB + j) % XBG
                        nc.tensor.matmul(gps[:, j, 0:2 * K2], xt_hi[:, bl_, :],
                                         sb["f160h"][:], start=True, stop=False)
                        nc.tensor.matmul(gps[:, j, 0:2 * K2], xt_lo[:, bl_, :],
                                         sb["f160l"][:], start=False, stop=True)
                    if g % 2 == 0:
                        gsb = gsbpool.tile([N1, TB, 2 * K2], F16, tag="gsb")
                    nc.scalar.copy(gsb[:, (g % 2) * GB:(g % 2 + 1) * GB, :],
                                   gps[:, :, 0:2 * K2])
                    if g % 2 == 0:
                        continue
                    blk = g // 2
                    bsl = slice(blk * TB, (blk + 1) * TB)
                    btr = bt[:, bsl, 0:K2]
                    bti = bt[:, bsl, K2:2 * K2]
                    gr = gsb[:, :, 0:K2]
                    gi = gsb[:, :, K2:2 * K2]
                    t1 = twpool.tile([N1, TB, K2], F16, tag="t1")
                    t2 = twpool.tile([N1, TB, K2], F16, tag="t2")
                    t3 = twpool.tile([N1, TB, K2], F16, tag="t3")
                    t4 = twpool.tile([N1, TB, K2], F16, tag="t4")
                    # btr = gr*tc + gi*ts (DVE); bti = gi*tc - gr*ts
                    # (bti pair alternates DVE/GPSIMD by block parity)
                    eng = nc.gpsimd if blk % 2 == 0 else nc.vector
                    nc.vector.tensor_tensor(t1[:], gr, tcb, ALU.mult)
                    nc.vector.tensor_tensor(t2[:], gi, tsb, ALU.mult)
                    nc.vector.tensor_tensor(btr, t1[:], t2[:], ALU.add)
                    nc.vector.tensor_tensor(t3[:], gi, tcb, ALU.mult)
                    eng.tensor_tensor(t4[:], gr, tsb, ALU.mult)
                    eng.tensor_tensor(bti, t3[:], t4[:], ALU.subtract)

            # ---- stage C + magnitude + layer-0 (fused; one act-table) ----
            with (
                tc.tile_pool(name="once", bufs=1) as once,
                tc.tile_pool(name="psum_acc", bufs=1, space="PSUM") as pacc,
            ):
                y0_ps = pacc.tile([BL, H0], F32, tag="yps")
                tr0_ps = pacc.tile([BL, H0 + 1], F32, tag="trps")
                # pass C1: stage-C MMs + Square + PE-fold + Sqrt -> AX
                # (act table: sqrt_and_others for Square/Sqrt)
                AX = once.tile([126, K2 // 2, BL], F16, tag="AX")
                NWG = NPAIR // WG
                wts = []
                with (
                    tc.tile_pool(name="wstream", bufs=4) as wpool,
                    tc.tile_pool(name="psum_x", bufs=3, space="PSUM") as px,
                    tc.tile_pool(name="psum_m2", bufs=2, space="PSUM") as pm2,
                    tc.tile_pool(name="sq", bufs=3) as sqpool,
                    tc.tile_pool(name="psb", bufs=2) as psbpool,
                    tc.tile_pool(name="hch", bufs=2) as hpool,
                ):
                    def stream_w0(gi_):
                        w0t = wpool.tile([126, WG * CK, H0], F16, tag="w0t")
                        a0t = wpool.tile([126, WG * CK, H0 + 1], F16, tag="a0t")
                        nc.sync.dma_start(
                            w0t[:], cd["w0stk"][:, gi_ * WG * CK:(gi_ + 1) * WG * CK, :])
                        nc.sync.dma_start(
                            a0t[:], cd["a0stk"][:, gi_ * WG * CK:(gi_ + 1) * WG * CK, :])
                        wts.append((w0t, a0t))

                    for gi_ in range(3):        # prefetch during C1
                        stream_w0(gi_)
                    for t in range(NPAIR):
                        m2p = pm2.tile([126, CK, BL], F32, tag="m2p")
                        for half in range(2):
                            cc = 2 * t + half
                            k2lo = cc * CK
                            P = px.tile([128, CK, BL], F32, tag="P")
                            nc.tensor.matmul(P[:], sb["W1s"][:],
                                             btv[:, k2lo:k2lo + CK, :],
                                             start=True, stop=False)
                            nc.tensor.matmul(P[:], sb["W2s"][:],
                                             btv[:, K2 + k2lo:K2 + k2lo + CK, :],
                                             start=False, stop=True)
                            sq = sqpool.tile([128, CK, BL], BF16, tag="sq")
                            if half == 0:
                                nc.scalar.activation(sq[:], P[:], AF.Square)
                            else:
                                # PSUM->SBUF copy (DVE) + square on idle GPSIMD
                                psb_ = psbpool.tile([128, CK, BL], BF16, tag="psb")
                                nc.vector.tensor_copy(psb_[:], P[:])
                                nc.gpsimd.tensor_tensor(sq[:], psb_[:], psb_[:],
                                                        ALU.mult)
                            # PE fold: m2pair[c] = sq[c] + sq[64+c]
                            fold = sb["folde"] if half == 0 else sb["foldo"]
                            nc.tensor.matmul(m2p[:], fold[:], sq[:],
                                             start=(half == 0), stop=(half == 1))
                        nc.scalar.activation(AX[:, t * CK:(t + 1) * CK, :], m2p[:],
                                             AF.Sqrt)

                    # Scheduling barrier C1->C2: `ones` reads every AX pair-slice,
                    # and every Ln uses it as its (==1.0) bias, so no Ln can be
                    # scheduled between C1 Sqrts (avoids sqrt<->ln act-table
                    # thrash: 2 table loads instead of 40).
                    axr = once.tile([126, 1], F32, tag="axr")
                    nc.vector.tensor_reduce(axr[:], AX[:, :, 0],
                                            axis=mybir.AxisListType.X, op=ALU.max)
                    ones = once.tile([126, 1], F32, tag="ones1")
                    nc.vector.tensor_scalar(ones[:], axr[:], 0.0, 1.0,
                                            op0=ALU.mult, op1=ALU.add)

                    # pass C2: log1p + h^2 + layer-0 matmuls (table: natural_log)
                    for t in range(NPAIR):
                        if t % WG == 0 and t // WG + 3 < NWG:
                            stream_w0(t // WG + 3)
                        w0t, a0t = wts[t // WG]
                        hch = hpool.tile([126, CK, BL], F16, tag="hch")
                        nc.scalar.activation(hch[:], AX[:, t * CK:(t + 1) * CK, :],
                                             AF.Ln, bias=ones[:])
                        hh = hpool.tile([126, CK, BL], F16, tag="hh")
                        nc.vector.tensor_tensor(hh[:], hch[:], hch[:], ALU.mult)
                        for j in range(CK):
                            ci = (t % WG) * CK + j
                            st = (t == 0 and j == 0)
                            sp = (t == NPAIR - 1 and j == CK - 1)
                            nc.tensor.matmul(y0_ps[:], hch[:, j, :], w0t[:, ci, :],
                                             start=st, stop=sp, skip_group_check=True)
                            nc.tensor.matmul(tr0_ps[:], hh[:, j, :], a0t[:, ci, :],
                                             start=st, stop=sp, skip_group_check=True)

                # ---- layer 0 epilogue ----
                rinv0 = _rsqrt(nc, once, tr0_ps[:, H0:H0 + 1], "rinv0")
                h1 = _layer_epilogue(nc, once, y0_ps[:], tr0_ps[:, 0:H0], rinv0[:],
                                     H0, sb["g0b"][:], sb["be0b"][:], sb["b0b"][:])

                # ---- layer 1 ----
                rv1 = _norm_rinv(nc, once, h1[:], H0)
                h1t = once.tile([128, 2, BL], F16, tag="h1t")
                for cidx in range(2):
                    pt = pacc.tile([128, BL], F32R, tag="trp")
                    nc.tensor.transpose(pt[:], h1[:, cidx * 128:(cidx + 1) * 128],
                                        sb["ident"][:])
                    nc.vector.tensor_copy(h1t[:, cidx, :], pt[:])
                hsq1 = once.tile([128, 2, BL], F16, tag="hsq1")
                nc.scalar.activation(hsq1[:], h1t[:], AF.Square)
                y1_ps = pacc.tile([BL, H1], F32, tag="yps")
                tr1_ps = pacc.tile([BL, H1], F32, tag="trps")
                for cidx in range(2):
                    nc.tensor.matmul(y1_ps[:], h1t[:, cidx, :], sb["w1t"][:, cidx, :],
                                     start=(cidx == 0), stop=(cidx == 1))
                    nc.tensor.matmul(tr1_ps[:], hsq1[:, cidx, :], sb["a1t"][:, cidx, :],
                                     start=(cidx == 0), stop=(cidx == 1))
                h2 = _layer_epilogue(nc, once, y1_ps[:], tr1_ps[:], rv1[:], H1,
                                     sb["g1b"][:], sb["be1b"][:], sb["b1b"][:])

                # ---- layer 2 ----
                rv2 = _norm_rinv(nc, once, h2[:], H1)
                h2p = pacc.tile([128, BL], F32R, tag="trp")
                nc.tensor.transpose(h2p[:], h2[:, 0:H1], sb["ident"][:])
                h2t = once.tile([128, BL], F16, tag="h2t")
                nc.vector.tensor_copy(h2t[:], h2p[:])
                hsq2 = once.tile([128, BL], F16, tag="hsq2")
                nc.scalar.activation(hsq2[:], h2t[:], AF.Square)
                y2_ps = pacc.tile([BL, H2], F32, tag="yps")
                tr2_ps = pacc.tile([BL, H2], F32, tag="trps")
                nc.tensor.matmul(y2_ps[:], h2t[:], sb["w2t"][:], start=True, stop=True)
                nc.tensor.matmul(tr2_ps[:], hsq2[:], sb["a2t"][:], start=True, stop=True)
                h3 = _layer_epilogue(nc, once, y2_ps[:], tr2_ps[:], rv2[:], H2,
                                     sb["g2b"][:], sb["be2b"][:], sb["b2b"][:])

                # ---- head ----
                h3p = pacc.tile([H2, BL], F32R, tag="trp")
                nc.tensor.transpose(h3p[:], h3[:, 0:H2], sb["ident"][:])
                h3t = once.tile([H2, BL], F16, tag="h3t")
                nc.vector.tensor_copy(h3t[:], h3p[:])
                hd_ps = pacc.tile([BL, 2], F32, tag="yps")
                nc.tensor.matmul(hd_ps[:], h3t[:], sb["hwt"][:], start=True, stop=True)
                osb = once.tile([BL, 1], F32, tag="osb")
                nc.scalar.activation(osb[:], hd_ps[:, 0:1], AF.Identity, bias=sb["hbb"][:])
                nc.sync.dma_start(out, osb[:])

    nc.compile()
    return nc


def make_in_maps(inputs):
    consts = build_consts(inputs)
    xfull = _f32(inputs["x"])
    in_maps = []
    for c in range(NCORES):
        m = dict(consts)
        xc = xfull[c * BL:(c + 1) * BL]
        m["xt"] = _f16(xc.reshape(BL, N2, N1).transpose(1, 0, 2))
        in_maps.append(m)
    return in_maps


def kernel(**inputs) -> np.ndarray:
    if "k" not in _cache:
        _cache["k"] = build_kernel()
    nc = _cache["k"]
    in_maps = make_in_maps(inputs)
    r = run_bass_kernel_spmd(nc, in_maps, core_ids=list(range(NCORES)))
    return np.concatenate([r.results[c]["out"][:, 0] for c in range(NCORES)], axis=0)


# revision 40
# speedup vs baseline: 1.1860x; 1.1860x over previous
"""Trainium2 Bass kernel for nn_LocalPlasticityNet (FFT front-end + Hebbian MLP).

Data-parallel over batch: 8 cores x 128 rows. Per core:
  FFT(20000) as four-step DFT, 20000 = 125*160, n = n1 + 125*n2,
  k = k2 + 160*k1 (k1 < 63 covers the needed half spectrum).
    stage A: G[n1,k2]  = sum_n2 xt[n2,n1] W160^{n2 k2}   (per-sample MMs, f16)
    stage B: Bt = T' * G twiddle                         (DVE + GPSIMD, f16)
    stage C: per k2-chunk, 2 MMs with packed stationaries
             W1=[c|0|-s], W2=[s|0|c] -> PSUM [Xr(0:63); Xi(64:127)]
  magnitude: ACT Square -> DVE add -> DVE pow(.5) -> ACT log1p  (single
  act-table region; chunk-PAIRS stacked on partitions 0:63 / 63:126)
  layer 0 streams W0/A0 (f16, host-permuted to the pair layout); the norm
  mask rides as column 256 of the A0 stream so sum(mask*h^2) accumulates in
  PSUM alongside the Hebbian trace. rsqrt everywhere via DVE pow(-0.5) (no
  sqrt-table loads). Layers 1/2 + head in f16/f32 on-chip.
x is host-side cast to f16 and pre-transposed to [n2, b, n1] so the DMA is
contiguous. Mean subtraction == zeroing the k=0 bin (W0/A0/mask rows f=0).
"""
import numpy as np
import ml_dtypes

import concourse.bass as bass
import concourse.tile as tile
import concourse.mybir as mybir
from concourse import bacc
from concourse.bass_utils import run_bass_kernel_spmd

AF = mybir.ActivationFunctionType
ALU = mybir.AluOpType
F32 = mybir.dt.float32
F32R = mybir.dt.float32r
F16 = mybir.dt.float16
BF16 = mybir.dt.bfloat16

B, N, NF = 1024, 20000, 10001
N1, N2 = 125, 160          # n = n1 + 125*n2
K1, K2 = 63, 160           # k = k2 + 160*k1
NCORES = 8
BL = B // NCORES           # 128
H0, H1, H2 = 256, 128, 64
LN_EPS = 1e-5
GB = 4                     # stage A batch group (psum banks)
XBG = 32                   # x streamed in batch groups of 32
CK = 4                     # stage C k2-chunk width
NPAIR = K2 // (2 * CK)     # 20 chunk pairs
WG = 2                     # W0 stream group: WG pairs per DMA

_cache = {}


def _f16(x):
    return np.ascontiguousarray(np.asarray(x, dtype=ml_dtypes.float16
                                           if hasattr(ml_dtypes, "float16")
                                           else np.float16))


def _f32(x):
    return np.ascontiguousarray(np.asarray(x, dtype=np.float32))


def build_consts(inputs):
    n2 = np.arange(N2)
    k2 = np.arange(K2)
    n1 = np.arange(N1)
    k1 = np.arange(K1)
    c = {}
    # stage A rhs [160, 320] = [cos | -sin] of 2pi n2 k2/160, split 128+32
    ang = 2 * np.pi * np.outer(n2, k2) / N2
    f160 = np.concatenate([np.cos(ang), -np.sin(ang)], axis=1)
    c["f160h"] = _f16(f160[0:128])
    c["f160l"] = _f16(f160[128:160])
    # twiddle [125, 160]
    angt = 2 * np.pi * np.outer(n1, k2) / N
    c["tc"] = _f16(np.cos(angt))
    c["ts"] = _f16(np.sin(angt))
    # stage C packed stationaries [125, 128]
    angc = 2 * np.pi * np.outer(n1, k1) / N1
    cc_ = np.cos(angc)
    ss_ = np.sin(angc)
    W1s = np.zeros((N1, 128))
    W1s[:, 0:63] = cc_
    W1s[:, 64:127] = -ss_
    W2s = np.zeros((N1, 128))
    W2s[:, 0:63] = ss_
    W2s[:, 64:127] = cc_
    c["W1s"] = _f16(W1s)
    c["W2s"] = _f16(W2s)
    # PE fold matrices: m2pair[c] = sq[c] + sq[64+c] (even chunk -> cols 0:63,
    # odd chunk -> cols 63:126)
    fe = np.zeros((128, 126))
    fo = np.zeros((128, 126))
    for r in range(63):
        fe[r, r] = 1.0
        fe[64 + r, r] = 1.0
        fo[r, 63 + r] = 1.0
        fo[64 + r, 63 + r] = 1.0
    c["folde"] = np.ascontiguousarray(fe.astype(ml_dtypes.bfloat16))
    c["foldo"] = np.ascontiguousarray(fo.astype(ml_dtypes.bfloat16))
    c["ident"] = _f32(np.eye(128))
    # layer 0 weights, permuted to the pair layout:
    # col c = 4t+j ; rows 0:63 -> f = 8t+j+160*k1 ; rows 63:126 -> f = 8t+4+j+160*k1
    W0 = np.asarray(inputs["W0"], np.float64)         # (256, 10001)
    al0 = np.asarray(inputs["alpha0"], np.float64)
    eta0 = float(inputs["eta0"])
    FP = K1 * K2
    W0p = np.zeros((FP, H0))
    W0p[:NF] = W0.T
    A0p = np.zeros((FP, H0))
    A0p[:NF] = (eta0 / (1.0 + np.exp(-al0))).T
    W0p[0] = 0.0          # f=0 bin acts as zero (mean subtraction)
    A0p[0] = 0.0
    maskf = np.zeros(FP)
    maskf[1:NF] = 1.0
    cs = np.arange(K2 // 2)
    tt, jj = cs // CK, cs % CK
    rr = np.arange(K1)
    f_top = (8 * tt + jj)[None, :] + 160 * rr[:, None]        # [63, 80]
    f_bot = (8 * tt + 4 + jj)[None, :] + 160 * rr[:, None]
    w0stk = np.zeros((126, K2 // 2, H0))
    w0stk[0:63] = W0p[f_top]
    w0stk[63:126] = W0p[f_bot]
    a0stk = np.zeros((126, K2 // 2, H0 + 1))
    a0stk[0:63, :, 0:H0] = A0p[f_top]
    a0stk[63:126, :, 0:H0] = A0p[f_bot]
    a0stk[0:63, :, H0] = maskf[f_top]
    a0stk[63:126, :, H0] = maskf[f_bot]
    c["w0stk"] = _f16(w0stk)
    c["a0stk"] = _f16(a0stk)
    c["b0b"] = _f32(np.tile(np.asarray(inputs["b0"], np.float32), (BL, 1)))
    # small layers: transposed [f, h] f16
    W1 = np.asarray(inputs["W1"], np.float64)         # (128, 256)
    A1 = float(inputs["eta1"]) / (1.0 + np.exp(-np.asarray(inputs["alpha1"], np.float64)))
    c["w1t"] = _f16(W1.T.reshape(2, 128, H1).transpose(1, 0, 2))   # [128, 2, 128]
    c["a1t"] = _f16(A1.T.reshape(2, 128, H1).transpose(1, 0, 2))
    W2 = np.asarray(inputs["W2"], np.float64)         # (64, 128)
    A2 = float(inputs["eta2"]) / (1.0 + np.exp(-np.asarray(inputs["alpha2"], np.float64)))
    c["w2t"] = _f16(W2.T)                                          # [128, 64]
    c["a2t"] = _f16(A2.T)
    hw = np.zeros((H2, 2))
    hw[:, 0] = np.asarray(inputs["head_w"], np.float64)[0]
    c["hwt"] = _f16(hw)    # [64, 2], col 1 zero-pad
    for nm, h in (("0", H0), ("1", H1), ("2", H2)):
        c[f"g{nm}b"] = _f32(np.tile(np.asarray(inputs[f"g{nm}"], np.float32), (BL, 1)))
        c[f"be{nm}b"] = _f32(np.tile(np.asarray(inputs[f"be{nm}"], np.float32), (BL, 1)))
    c["b1b"] = _f32(np.tile(np.asarray(inputs["b1"], np.float32), (BL, 1)))
    c["b2b"] = _f32(np.tile(np.asarray(inputs["b2"], np.float32), (BL, 1)))
    c["hbb"] = _f32(np.tile(np.asarray(inputs["head_b"], np.float32).reshape(1), (BL, 1)))
    return c


CONST_DTYPES = {
    "f160h": F16, "f160l": F16, "tc": F16, "ts": F16,
    "W1s": F16, "W2s": F16, "folde": BF16, "foldo": BF16, "ident": F32R,
    "w0stk": F16, "a0stk": F16, "b0b": F32,
    "w1t": F16, "a1t": F16, "w2t": F16, "a2t": F16, "hwt": F16,
    "g0b": F32, "be0b": F32, "g1b": F32, "be1b": F32, "g2b": F32, "be2b": F32,
    "b1b": F32, "b2b": F32, "hbb": F32,
}

SHAPES = {
    "f160h": [128, 2 * K2], "f160l": [32, 2 * K2],
    "tc": [N1, K2], "ts": [N1, K2],
    "W1s": [N1, 128], "W2s": [N1, 128],
    "folde": [128, 126], "foldo": [128, 126], "ident": [128, 128],
    "w0stk": [126, K2 // 2, H0], "a0stk": [126, K2 // 2, H0 + 1], "b0b": [BL, H0],
    "w1t": [128, 2, H1], "a1t": [128, 2, H1],
    "w2t": [H1, H2], "a2t": [H1, H2], "hwt": [H2, 2],
    "g0b": [BL, H0], "be0b": [BL, H0], "g1b": [BL, H1], "be1b": [BL, H1],
    "g2b": [BL, H2], "be2b": [BL, H2], "b1b": [BL, H1], "b2b": [BL, H2],
    "hbb": [BL, 1],
}


I32 = mybir.dt.int32


def _rsqrt(nc, once, src_ap, tag, add=None):
    """[BL,1] rsqrt via DVE quake-seed + 2 Newton iterations (no act table)."""
    v = once.tile([BL, 1], F32, tag=f"{tag}v")
    if add is not None:
        nc.vector.tensor_scalar_add(v[:], src_ap, add)
    else:
        nc.vector.tensor_copy(v[:], src_ap)
    iv = once.tile([BL, 1], I32, tag=f"{tag}i")
    nc.vector.tensor_scalar(iv[:], v[:].bitcast(I32), 1, None,
                            op0=ALU.logical_shift_right)
    nc.vector.tensor_scalar(iv[:], iv[:], -1, 0x5f3759df,
                            op0=ALU.mult, op1=ALU.add)
    y = once.tile([BL, 1], F32, tag=f"{tag}y")
    nc.vector.tensor_copy(y[:], iv[:].bitcast(F32))
    t1 = once.tile([BL, 1], F32, tag=f"{tag}t")
    for _ in range(2):
        nc.vector.tensor_tensor(t1[:], y[:], y[:], ALU.mult)
        nc.vector.tensor_tensor(t1[:], t1[:], v[:], ALU.mult)
        nc.vector.tensor_scalar(t1[:], t1[:], -0.5, 1.5, op0=ALU.mult, op1=ALU.add)
        nc.vector.tensor_tensor(y[:], y[:], t1[:], ALU.mult)
    return y


def _layer_epilogue(nc, once, y_ps, tr_ps, rinv, H, gb, beb, bb):
    """y = y_slow + tanh(y_slow) * (trace * rinv); LayerNorm(g,be); exact GELU.
    ACT funcs used: Tanh / Gelu / Identity -> one act table; LN stats via
    DVE bn_stats, rstd via DVE pow(-0.5)."""
    ysb = once.tile([BL, H], F32, tag=f"ysb{H}")
    if bb is not None:
        nc.vector.tensor_tensor(ysb[:], y_ps, bb, ALU.add)
    else:
        nc.vector.tensor_copy(ysb[:], y_ps)
    tnh = once.tile([BL, H], F32, tag=f"tnh{H}")
    nc.scalar.activation(tnh[:], ysb[:], AF.Tanh)
    pl = once.tile([BL, H], F32, tag=f"pl{H}")
    nc.vector.scalar_tensor_tensor(pl[:], tr_ps, rinv, tnh[:], ALU.mult, ALU.mult)
    y = once.tile([BL, H], F32, tag=f"y{H}")
    nc.vector.tensor_tensor(y[:], ysb[:], pl[:], ALU.add)
    # LayerNorm via bn_stats (mean+var in one pass)
    stats = once.tile([BL, 6], F32, tag=f"st{H}")
    nc.vector.bn_stats(stats[:], y[:])
    mv = once.tile([BL, 2], F32, tag=f"mv{H}")
    nc.vector.bn_aggr(mv[:], stats[:])
    nmu = once.tile([BL, 1], F32, tag=f"nmu{H}")
    nc.vector.tensor_scalar_mul(nmu[:], mv[:, 0:1], -1.0)
    rstd = _rsqrt(nc, once, mv[:, 1:2], f"rstd{H}", add=LN_EPS)
    ty = once.tile([BL, H], F32, tag=f"ty{H}")
    nc.scalar.activation(ty[:], y[:], AF.Identity, bias=nmu[:])
    yn = once.tile([BL, H], F32, tag=f"yn{H}")
    nc.vector.scalar_tensor_tensor(yn[:], ty[:], rstd[:], gb, ALU.mult, ALU.mult)
    nc.vector.tensor_tensor(yn[:], yn[:], beb, ALU.add)
    hn = once.tile([BL, H], F32R, tag=f"hn{H}")
    nc.scalar.activation(hn[:], yn[:], AF.Gelu)
    return hn


def _norm_rinv(nc, once, h, H):
    """rinv[b,1] ~= 1/||h_row||_2 via Square-accum + DVE pow(-0.5)."""
    d = once.tile([BL, H], F32, tag=f"nsq{H}")
    ns = once.tile([BL, 1], F32, tag=f"nss{H}")
    nc.scalar.activation(d[:], h, AF.Square, accum_out=ns[:])
    return _rsqrt(nc, once, ns[:], f"nrv{H}")


def build_kernel(reps: int = 1):
    nc = bacc.Bacc("TRN2", target_bir_lowering=False, debug=False, num_devices=1)
    xtd = nc.dram_tensor("xt", [N2, BL, N1], F16, kind="ExternalInput").ap()
    cd = {nm: nc.dram_tensor(nm, shp, CONST_DTYPES[nm], kind="ExternalInput").ap()
          for nm, shp in SHAPES.items()}
    out = nc.dram_tensor("out", [BL, 1], F32, kind="ExternalOutput").ap()

    import contextlib
    with tile.TileContext(nc) as tc:
        rep_ctx = tc.For_i(0, reps, 1) if reps > 1 else contextlib.nullcontext()
        with (
            rep_ctx,
            tc.tile_pool(name="const", bufs=1) as cpool,
            tc.tile_pool(name="bt", bufs=1) as btpool,
            tc.tile_pool(name="xt", bufs=2) as xpool,
        ):
            # prefetch the first x block before anything else hits the DMA queue
            xt_hi0 = xpool.tile([128, XBG, N1], F16, tag="xthi")
            xt_lo0 = xpool.tile([32, XBG, N1], F16, tag="xtlo")
            nc.sync.dma_start(xt_hi0[:], xtd[0:128, 0:XBG, :])
            nc.sync.dma_start(xt_lo0[:], xtd[128:160, 0:XBG, :])
            # ---- resident constants (stage-A-critical first) ----
            sb = {}
            for nm in ("f160h", "f160l", "tc", "ts", "W1s", "W2s",
                       "folde", "foldo", "ident",
                       "w1t", "a1t", "w2t", "a2t", "hwt",
                       "g0b", "be0b", "g1b", "be1b", "g2b", "be2b",
                       "b0b", "b1b", "b2b", "hbb"):
                t = cpool.tile(SHAPES[nm], CONST_DTYPES[nm], tag=nm)
                nc.sync.dma_start(t[:], cd[nm])
                sb[nm] = t

            bt = btpool.tile([N1, BL, 2 * K2], F16, tag="bt")  # [n1, b, Btr|Bti]
            btv = bt.rearrange("p b k -> p k b")               # k2-major view
            TB = 2 * GB                                        # twiddle block: 8 samples
            tcb = sb["tc"][:, None, :].to_broadcast((N1, TB, K2))
            tsb = sb["ts"][:, None, :].to_broadcast((N1, TB, K2))

            # ---- stage A + twiddle ----
            with (
                tc.tile_pool(name="psum_g", bufs=2, space="PSUM") as pg,
                tc.tile_pool(name="gsb", bufs=3) as gsbpool,
                tc.tile_pool(name="tw", bufs=3) as twpool,
            ):
                for g in range(BL // GB):
                    if g % (XBG // GB) == 0:
                        bg = g * GB // XBG
                        if bg == 0:
                            xt_hi, xt_lo = xt_hi0, xt_lo0
                        else:
                            xt_hi = xpool.tile([128, XBG, N1], F16, tag="xthi")
                            xt_lo = xpool.tile([32, XBG, N1], F16, tag="xtlo")
                            nc.sync.dma_start(xt_hi[:], xtd[0:128, bg * XBG:(bg + 1) * XBG, :])
                            nc.sync.dma_start(xt_lo[:], xtd[128:160, bg * XBG:(bg + 1) * XBG, :])
                    gps = pg.tile([N1, GB, 512], F32, tag="gps")
                    for j in range(GB):
                        bl_ = (g * GB + j) % XBG
                        nc.tensor.matmul(gps[:, j, 0:2 * K2], xt_hi[:, bl_, :],
                                         sb["f160h"][:], start=True, stop=False)
                        nc.tensor.matmul(gps[:, j, 0:2 * K2], xt_lo[:, bl_, :],
                                         sb["f160l"][:], start=False, stop=True)
                    if g % 2 == 0:
                        gsb = gsbpool.tile([N1, TB, 2 * K2], F16, tag="gsb")
                    nc.scalar.copy(gsb[:, (g % 2) * GB:(g % 2 + 1) * GB, :],
                                   gps[:, :, 0:2 * K2])
                    if g % 2 == 0:
                        continue
                    blk = g // 2
                    bsl = slice(blk * TB, (blk + 1) * TB)
                    btr = bt[:, bsl, 0:K2]
                    bti = bt[:, bsl, K2:2 * K2]
                    gr = gsb[:, :, 0:K2]
                    gi = gsb[:, :, K2:2 * K2]
                    t1 = twpool.tile([N1, TB, K2], F16, tag="t1")
                    t2 = twpool.tile([N1, TB, K2], F16, tag="t2")
                    t3 = twpool.tile([N1, TB, K2], F16, tag="t3")
                    t4 = twpool.tile([N1, TB, K2], F16, tag="t4")
                    # btr = gr*tc + gi*ts (DVE); bti = gi*tc - gr*ts
                    # (bti pair on GPSIMD for 10/16 blocks: HW A/B showed the
                    # pool engine outperforms its cost model here)
                    eng = nc.gpsimd if blk % 8 < 5 else nc.vector
                    nc.vector.tensor_tensor(t1[:], gr, tcb, ALU.mult)
                    nc.vector.tensor_tensor(t2[:], gi, tsb, ALU.mult)
                    nc.vector.tensor_tensor(btr, t1[:], t2[:], ALU.add)
                    nc.vector.tensor_tensor(t3[:], gi, tcb, ALU.mult)
                    eng.tensor_tensor(t4[:], gr, tsb, ALU.mult)
                    eng.tensor_tensor(bti, t3[:], t4[:], ALU.subtract)

            # ---- stage C + magnitude + layer-0 (fused; one act-table) ----
            with (
                tc.tile_pool(name="once", bufs=1) as once,
                tc.tile_pool(name="psum_acc", bufs=1, space="PSUM") as pacc,
            ):
                y0_ps = pacc.tile([BL, H0], F32, tag="yps")
                tr0_ps = pacc.tile([BL, H0 + 1], F32, tag="trps")
                # pass C1: stage-C MMs + Square + PE-fold + Sqrt -> AX
                # (act table: sqrt_and_others for Square/Sqrt)
                AX = once.tile([126, K2 // 2, BL], F16, tag="AX")
                NWG = NPAIR // WG
                wts = []
                with (
                    tc.tile_pool(name="wstream", bufs=5) as wpool,
                    tc.tile_pool(name="psum_x", bufs=3, space="PSUM") as px,
                    tc.tile_pool(name="psum_m2", bufs=2, space="PSUM") as pm2,
                    tc.tile_pool(name="sq", bufs=3) as sqpool,
                    tc.tile_pool(name="psb", bufs=2) as psbpool,
                    tc.tile_pool(name="hch", bufs=2) as hpool,
                ):
                    def stream_w0(gi_):
                        w0t = wpool.tile([126, WG * CK, H0], F16, tag="w0t")
                        a0t = wpool.tile([126, WG * CK, H0 + 1], F16, tag="a0t")
                        nc.sync.dma_start(
                            w0t[:], cd["w0stk"][:, gi_ * WG * CK:(gi_ + 1) * WG * CK, :])
                        nc.sync.dma_start(
                            a0t[:], cd["a0stk"][:, gi_ * WG * CK:(gi_ + 1) * WG * CK, :])
                        wts.append((w0t, a0t))

                    for gi_ in range(4):        # prefetch during C1
                        stream_w0(gi_)
                    for t in range(NPAIR):
                        m2p = pm2.tile([126, CK, BL], F32, tag="m2p")
                        k2e = 2 * t * CK
                        k2o = k2e + CK
                        # W1s/W2s each loaded once per pair (stationary reuse)
                        P_e = px.tile([128, CK, BL], F32, tag="P")
                        P_o = px.tile([128, CK, BL], F32, tag="P")
                        nc.tensor.matmul(P_e[:], sb["W1s"][:],
                                         btv[:, k2e:k2e + CK, :],
                                         start=True, stop=False)
                        nc.tensor.matmul(P_o[:], sb["W1s"][:],
                                         btv[:, k2o:k2o + CK, :],
                                         start=True, stop=False)
                        nc.tensor.matmul(P_e[:], sb["W2s"][:],
                                         btv[:, K2 + k2e:K2 + k2e + CK, :],
                                         start=False, stop=True)
                        nc.tensor.matmul(P_o[:], sb["W2s"][:],
                                         btv[:, K2 + k2o:K2 + k2o + CK, :],
                                         start=False, stop=True)
                        sq_e = sqpool.tile([128, CK, BL], BF16, tag="sq")
                        nc.scalar.activation(sq_e[:], P_e[:], AF.Square)
                        # odd square: PSUM->SBUF copy (DVE) + square on GPSIMD
                        psb_ = psbpool.tile([128, CK, BL], BF16, tag="psb")
                        nc.vector.tensor_copy(psb_[:], P_o[:])
                        sq_o = sqpool.tile([128, CK, BL], BF16, tag="sq")
                        nc.gpsimd.tensor_tensor(sq_o[:], psb_[:], psb_[:], ALU.mult)
                        # PE fold: m2pair[c] = sq[c] + sq[64+c]
                        nc.tensor.matmul(m2p[:], sb["folde"][:], sq_e[:],
                                         start=True, stop=False)
                        nc.tensor.matmul(m2p[:], sb["foldo"][:], sq_o[:],
                                         start=False, stop=True)
                        nc.scalar.activation(AX[:, t * CK:(t + 1) * CK, :], m2p[:],
                                             AF.Sqrt)

                    # Scheduling barrier C1->C2: `ones` reads every AX pair-slice,
                    # and every Ln uses it as its (==1.0) bias, so no Ln can be
                    # scheduled between C1 Sqrts (avoids sqrt<->ln act-table
                    # thrash: 2 table loads instead of 40).
                    axr = once.tile([126, 1], F32, tag="axr")
                    nc.vector.tensor_reduce(axr[:], AX[:, :, 0],
                                            axis=mybir.AxisListType.X, op=ALU.max)
                    ones = once.tile([126, 1], F32, tag="ones1")
                    nc.vector.tensor_scalar(ones[:], axr[:], 0.0, 1.0,
                                            op0=ALU.mult, op1=ALU.add)

                    # pass C2: log1p + h^2 + layer-0 matmuls (table: natural_log)
                    for t in range(NPAIR):
                        if t % WG == 0 and t // WG + 4 < NWG:
                            stream_w0(t // WG + 4)
                        w0t, a0t = wts[t // WG]
                        hch = hpool.tile([126, CK, BL], F16, tag="hch")
                        nc.scalar.activation(hch[:], AX[:, t * CK:(t + 1) * CK, :],
                                             AF.Ln, bias=ones[:])
                        hh = hpool.tile([126, CK, BL], F16, tag="hh")
                        nc.vector.tensor_tensor(hh[:], hch[:], hch[:], ALU.mult)
                        for j in range(CK):
                            ci = (t % WG) * CK + j
                            st = (t == 0 and j == 0)
                            sp = (t == NPAIR - 1 and j == CK - 1)
                            nc.tensor.matmul(y0_ps[:], hch[:, j, :], w0t[:, ci, :],
                                             start=st, stop=sp, skip_group_check=True)
                            nc.tensor.matmul(tr0_ps[:], hh[:, j, :], a0t[:, ci, :],
                                             start=st, stop=sp, skip_group_check=True)

                # ---- layer 0 epilogue ----
                rinv0 = _rsqrt(nc, once, tr0_ps[:, H0:H0 + 1], "rinv0")
                h1 = _layer_epilogue(nc, once, y0_ps[:], tr0_ps[:, 0:H0], rinv0[:],
                                     H0, sb["g0b"][:], sb["be0b"][:], sb["b0b"][:])

                # ---- layer 1 ----
                rv1 = _norm_rinv(nc, once, h1[:], H0)
                h1t = once.tile([128, 2, BL], F16, tag="h1t")
                for cidx in range(2):
                    pt = pacc.tile([128, BL], F32R, tag="trp")
                    nc.tensor.transpose(pt[:], h1[:, cidx * 128:(cidx + 1) * 128],
                                        sb["ident"][:])
                    nc.vector.tensor_copy(h1t[:, cidx, :], pt[:])
                hsq1 = once.tile([128, 2, BL], F16, tag="hsq1")
                nc.scalar.activation(hsq1[:], h1t[:], AF.Square)
                y1_ps = pacc.tile([BL, H1], F32, tag="yps")
                tr1_ps = pacc.tile([BL, H1], F32, tag="trps")
                for cidx in range(2):
                    nc.tensor.matmul(y1_ps[:], h1t[:, cidx, :], sb["w1t"][:, cidx, :],
                                     start=(cidx == 0), stop=(cidx == 1))
                    nc.tensor.matmul(tr1_ps[:], hsq1[:, cidx, :], sb["a1t"][:, cidx, :],
                                     start=(cidx == 0), stop=(cidx == 1))
                h2 = _layer_epilogue(nc, once, y1_ps[:], tr1_ps[:], rv1[:], H1,
                                     sb["g1b"][:], sb["be1b"][:], sb["b1b"][:])

                # ---- layer 2 ----
                rv2 = _norm_rinv(nc, once, h2[:], H1)
                h2p = pacc.tile([128, BL], F32R, tag="trp")
                nc.tensor.transpose(h2p[:], h2[:, 0:H1], sb["ident"][:])
                h2t = once.tile([128, BL], F16, tag="h2t")
                nc.vector.tensor_copy(h2t[:], h2p[:])
                hsq2 = once.tile([128, BL], F16, tag="hsq2")
                nc.scalar.activation(hsq2[:], h2t[:], AF.Square)
                y2_ps = pacc.tile([BL, H2], F32, tag="yps")
                tr2_ps = pacc.tile([BL, H2], F32, tag="trps")
                nc.tensor.matmul(y2_ps[:], h2t[:], sb["w2t"][:], start=True, stop=True)
                nc.tensor.matmul(tr2_ps[:], hsq2[:], sb["a2t"][:], start=True, stop=True)
                h3 = _layer_epilogue(nc, once, y2_ps[:], tr2_ps[:], rv2[:], H2,
                                     sb["g2b"][:], sb["be2b"][:], sb["b2b"][:])

                # ---- head ----
                h3p = pacc.tile([H2, BL], F32R, tag="trp")
                nc.tensor.transpose(h3p[:], h3[:, 0:H2], sb["ident"][:])
                h3t = once.tile([H2, BL], F16, tag="h3t")
                nc.vector.tensor_copy(h3t[:], h3p[:])
                hd_ps = pacc.tile([BL, 2], F32, tag="yps")
                nc.tensor.matmul(hd_ps[:], h3t[:], sb["hwt"][:], start=True, stop=True)
                osb = once.tile([BL, 1], F32, tag="osb")
                nc.scalar.activation(osb[:], hd_ps[:, 0:1], AF.Identity, bias=sb["hbb"][:])
                nc.sync.dma_start(out, osb[:])

    nc.compile()
    return nc


def make_in_maps(inputs):
    consts = build_consts(inputs)
    xfull = _f32(inputs["x"])
    in_maps = []
    for c in range(NCORES):
        m = dict(consts)
        xc = xfull[c * BL:(c + 1) * BL]
        m["xt"] = _f16(xc.reshape(BL, N2, N1).transpose(1, 0, 2))
        in_maps.append(m)
    return in_maps


def kernel(**inputs) -> np.ndarray:
    if "k" not in _cache:
        _cache["k"] = build_kernel()
    nc = _cache["k"]
    in_maps = make_in_maps(inputs)
    r = run_bass_kernel_spmd(nc, in_maps, core_ids=list(range(NCORES)))
    return np.concatenate([r.results[c]["out"][:, 0] for c in range(NCORES)], axis=0)
